# revision 35
# baseline (speedup 1.0000x reference)
"""KAN layer (Chebyshev order-7 on tanh(x)) as a Bass/Tile TRN2 kernel.

Math: out[b,o] = sum_{i,k} T_k(tanh(x[b,i])) * W[o,i,k] + bias[o],  k=0..7.

T_0 == 1, so the k=0 weight slice folds into an effective bias on the host:
bias_eff[o] = bias[o] + sum_i W[o,i,0]. The device contracts over the
remaining 7*1024 = 7168 (i,k) pairs.

Sharding: data-parallel over batch. Each of the 8 cores takes 512 batch
rows; every core holds the full weights. Per core this is a
[7168 x 512] basis (built on-chip from x) against [7168 x 1024] weights,
accumulated as out.T tiles [128(o) x 512(b)] across 8 PSUM banks.

Layout/scheduling (cost-model time ~105.1us vs a 95.6us PE floor;
fp32 baseline was 117.9us):
- Weights are cast to bf16 on the host (rel tolerance is 2e-2; bf16
  weight+basis quantization lands ~2e-3) and laid out [it, p, k*o] so each
  i-tile is ONE [128 x 7168] DMA (14336B per partition line). This halves
  HBM traffic and cuts DMA count ~5x (HWDGE serialization and SP sequencer
  issue time were co-bottlenecks at fp32).
- i-tile 0's weights are fetched as 7 per-k slices so the first matmul
  doesn't wait on a 5us transfer.
- x0 goes out FIRST on the SP queue (ahead of weights) to win the
  serialized HWDGE+DMA device; x1..x7 stream via gpsimd/SWDGE in
  parallel. bf16 matmul operands, all 8 PSUM banks accumulating.
- PE p-state ramps from the FIRST PE op (gaps don't reset it): 8 zeroing
  matmuls (0x0, start=True) run during the startup window, so the banks
  are pre-zeroed (all real matmuls accumulate with start=False -- a
  column-split start=True would wipe its sibling half: start resets the
  whole bank) and the 2.4GHz ramp completes before real work arrives.
- Chebyshev chain in fp32 on DVE (production rate 7.1us/i-tile vs PE
  consumption 11.9us/i-tile), bf16 basis copies on Act (Tanh/Copy share
  one activation table - no reloads). T_1 of i-tile 0 comes from a
  second bf16-output tanh so the first matmul skips the cast latency.
- Final i-tile runs ot-major so the 8 PSUM banks stop staggered ~1.5us
  apart; per-ot epilogue (bias add on Act for even ot / DVE for odd ot,
  store DMA on SP for even / gpsimd for odd) overlaps the tail matmuls.
  The globally-last otile uses the faster Act+SP chain.
- bf16 output (host casts back to f32): halves store traffic and the
  final store's latency chain.
"""

import sys

sys.path.insert(0, "/opt/trn_rl_repo")

import ml_dtypes
import numpy as np

import concourse.bass as bass  # noqa: F401  (engine types come via bacc)
import concourse.mybir as mybir
from concourse import bacc
from concourse.bass_utils import run_bass_kernel_spmd
from concourse.tile import TileContext

P = 128
N_CORES = 8
BATCH = 4096
B_CORE = BATCH // N_CORES  # 512
IN_F = 1024
OUT_F = 1024
KORD = 7  # Chebyshev T_1..T_7 (T_0 folded into bias)
N_ITILES = IN_F // P  # 8
N_OTILES = OUT_F // P  # 8
NSTEPS = N_ITILES * KORD  # 56 contraction steps of K=128
WLINE = KORD * OUT_F  # 7168 weight elems per partition line per i-tile

F32 = mybir.dt.float32
BF16 = mybir.dt.bfloat16
ACT_COPY = mybir.ActivationFunctionType.Copy
ACT_TANH = mybir.ActivationFunctionType.Tanh
ACT_IDENT = mybir.ActivationFunctionType.Identity
MULT = mybir.AluOpType.mult

_NC_CACHE = None


def _build():
    """Build + compile the single-core Bass program (SPMD across 8 cores)."""
    global _NC_CACHE
    if _NC_CACHE is not None:
        return _NC_CACHE

    nc = bacc.Bacc("TRN2", target_bir_lowering=False, debug=False)

    # xT[i, b] = x[b, i] for this core's batch slice.
    xT = nc.declare_dram_parameter("xT", [IN_F, B_CORE], F32, isOutput=False)
    # wT[it, p, k*1024 + o] = weights[o, it*128+p, k+1]  (bf16).
    wT = nc.declare_dram_parameter("wT", [N_ITILES, P, WLINE], BF16, isOutput=False)
    # biasT[p, ot] = bias_eff[ot*128 + p]
    biasT = nc.declare_dram_parameter("biasT", [P, N_OTILES], F32, isOutput=False)
    # bf16 output: halves store traffic and the final store latency chain;
    # output quantization (~0.008 absmax on a 2.33 absmax signal) is well
    # inside the 2e-2 gate. The host casts back to f32.
    outT = nc.declare_dram_parameter("outT", [OUT_F, B_CORE], BF16, isOutput=True)

    with TileContext(nc) as tc:
        with (
            tc.tile_pool(name="xraw", bufs=1) as xraw_pool,
            tc.tile_pool(name="w0", bufs=1) as w0_pool,
            tc.tile_pool(name="wbig", bufs=3) as wbig_pool,
            tc.tile_pool(name="basis", bufs=1) as basis_pool,
            tc.tile_pool(name="chain", bufs=8) as chain_pool,
            tc.tile_pool(name="tmp", bufs=3) as tmp_pool,
            tc.tile_pool(name="osb", bufs=8) as osb_pool,
            tc.tile_pool(name="misc", bufs=1) as misc_pool,
            tc.tile_pool(name="psum", bufs=1, space="PSUM") as psum_pool,
        ):
            # ---- warmup operand: one small DVE memset (DVE is idle until
            # the first chain op ~4.5us in; gpsimd must not be delayed, it
            # streams x).
            dummy_z = misc_pool.tile([P, P], BF16, name="dummy_z")
            nc.vector.memset(dummy_z, 0.0)

            # ---- input DMAs ----
            # x0 heads the critical path (x0 -> tanh -> first matmul): it
            # goes out FIRST on the SP queue, ahead of the weight DMAs, so
            # it wins the serialized HWDGE + DMA device. x1..x7 stream via
            # the gpsimd/SWDGE queue.
            xraw = []
            for it in range(N_ITILES):
                xr = xraw_pool.tile([P, B_CORE], F32, name=f"x_{it}")
                if it == 0:
                    nc.sync.dma_start(out=xr, in_=xT[:P, :])
                else:
                    nc.gpsimd.dma_start(out=xr, in_=xT[it * P : (it + 1) * P, :])
                xraw.append(xr)

            # Weights on the SP/HWDGE queue (behind x0). i-tiles 0 and 1
            # split per-k: the serialized DMA device must meet per-step
            # deadlines (one k-slice every 1.71us of PE work from ~4.6us on)
            # and a monolithic 5.1us i-tile transfer can't be scheduled
            # around them. i-tiles 2..7 have enough slack for big DMAs.
            wsmall = {}
            for it in range(2):
                for k in range(KORD):
                    wk = w0_pool.tile([P, OUT_F], BF16, name=f"w{it}_{k}")
                    nc.sync.dma_start(
                        out=wk, in_=wT[it, :, k * OUT_F : (k + 1) * OUT_F]
                    )
                    wsmall[(it, k)] = wk
            wbig = [None] * N_ITILES
            for it in range(2, N_ITILES):
                wb = wbig_pool.tile([P, WLINE], BF16, tag="wbig")
                nc.sync.dma_start(out=wb, in_=wT[it, :, :])
                wbig[it] = wb
            bias_sb = misc_pool.tile([P, N_OTILES], F32, name="bias_sb")
            nc.sync.dma_start(out=bias_sb, in_=biasT[:, :])

            # ---- Chebyshev basis: fp32 chain on DVE, bf16 copies on Act ----
            basis = []  # basis[it][k-1] = T_k(tanh(x tile it)) as [128,512] bf16
            for it in range(N_ITILES):
                t = chain_pool.tile([P, B_CORE], F32, tag="chain")
                tiles = []
                b0 = basis_pool.tile([P, B_CORE], BF16, name=f"b_{it}_0")
                if it == 0:
                    # bf16 tanh first, and the fp32 chain value is DERIVED
                    # from it (t = Copy(b0)): the data dependency forces the
                    # scheduler to run the bf16 tanh first, so the matmul
                    # stream starts at ~4.55us instead of 5.08us (a second
                    # independent tanh gets reordered after the fp32 one).
                    # T_2 comes straight off the bf16 tanh on DVE (2t^2-1)
                    # so the chain latency never gates step (0,1). Chain
                    # from bf16-quantized tanh costs ~2e-3 rel err on this
                    # i-tile's terms (measured 7.9e-3 if ALL i-tiles did it;
                    # gate is 2e-2).
                    nc.scalar.activation(b0, xraw[it], ACT_TANH)
                    nc.scalar.activation(t, b0, ACT_COPY)
                    b1 = basis_pool.tile([P, B_CORE], BF16, name="b_0_1")
                    tsq = tmp_pool.tile([P, B_CORE], BF16, tag="tmp")
                    nc.vector.tensor_mul(tsq, b0, b0)
                    nc.vector.tensor_scalar(
                        b1, tsq, 2.0, -1.0, MULT, mybir.AluOpType.add
                    )
                else:
                    nc.scalar.activation(t, xraw[it], ACT_TANH)
                    nc.scalar.activation(b0, t, ACT_COPY)
                tiles.append(b0)

                prev, prev2 = t, None
                for k in range(2, KORD + 1):
                    tmp = tmp_pool.tile([P, B_CORE], F32, tag="tmp")
                    # tmp = (t * 2) * T_{k-1}
                    nc.vector.scalar_tensor_tensor(
                        out=tmp, in0=t, scalar=2.0, in1=prev, op0=MULT, op1=MULT
                    )
                    cur = chain_pool.tile([P, B_CORE], F32, tag="chain")
                    if k == 2:
                        nc.vector.tensor_scalar_sub(cur, tmp, 1.0)
                    else:
                        nc.vector.tensor_sub(cur, tmp, prev2)
                    if it == 0 and k == 2:
                        tiles.append(b1)  # already produced via the direct path
                    else:
                        bk = basis_pool.tile([P, B_CORE], BF16, name=f"b_{it}_{k - 1}")
                        nc.scalar.activation(bk, cur, ACT_COPY)
                        tiles.append(bk)
                    prev2, prev = prev, cur
                basis.append(tiles)

            # ---- Matmul accumulation: out.T[ot] += w_s[:, ot].T @ basis_s ----
            psums = [
                psum_pool.tile([P, B_CORE], F32, name=f"ps_{ot}")
                for ot in range(N_OTILES)
            ]

            def wslice(it, k, ot):
                if it < 2:
                    return wsmall[(it, k)][:, ot * P : (ot + 1) * P]
                col = k * OUT_F + ot * P
                return wbig[it][:, col : col + P]

            # Warmup/zero stream: 32 chunked 0x0 matmuls (start=True zeroes
            # the whole addressed bank) spanning ~1.2us to ~4.6us. The span
            # matters, not just the zeroing: the PE p-state ramp runs from
            # the first PE op, BUT a multi-us idle gap before the real
            # stream resets it (measured: 1.65us gap is safe, 3.6us is not)
            # -- so the warmup stream must end right where the real stream
            # begins. All real matmuls then accumulate with start=False.
            for c in range(4):
                for ot in range(N_OTILES):
                    nc.tensor.matmul(
                        psums[ot][:, c * P : (c + 1) * P],
                        lhsT=dummy_z,
                        rhs=dummy_z,
                        start=True,
                        stop=False,
                        skip_group_check=True,
                    )

            # i-tiles 0..6: k-major, all 8 PSUM banks in flight.
            for it in range(N_ITILES - 1):
                for k in range(KORD):
                    for ot in range(N_OTILES):
                        nc.tensor.matmul(
                            psums[ot],
                            lhsT=wslice(it, k, ot),
                            rhs=basis[it][k],
                            start=False,
                            stop=False,
                            skip_group_check=True,
                        )

            # Final i-tile: ot-major so banks stop staggered; epilogue per ot
            # overlaps the remaining matmuls. Odd ots (DVE bias + gpsimd
            # store, the slower chain) go first; the globally-last ot (6)
            # uses the faster Act-bias + SP/HWDGE store path.
            last = N_ITILES - 1
            for ot in (1, 3, 5, 7, 0, 2, 4, 6):
                for k in range(KORD):
                    nc.tensor.matmul(
                        psums[ot],
                        lhsT=wslice(last, k, ot),
                        rhs=basis[last][k],
                        start=False,
                        stop=(k == KORD - 1),
                        skip_group_check=True,
                    )
                osb = osb_pool.tile([P, B_CORE], BF16, tag="osb")
                if ot % 2 == 0:
                    nc.scalar.activation(
                        osb, psums[ot], ACT_IDENT,
                        bias=bias_sb[:, ot : ot + 1], scale=1.0,
                    )
                    nc.sync.dma_start(out=outT[ot * P : (ot + 1) * P, :], in_=osb)
                else:
                    nc.vector.tensor_scalar_add(
                        osb, psums[ot], bias_sb[:, ot : ot + 1]
                    )
                    nc.gpsimd.dma_start(out=outT[ot * P : (ot + 1) * P, :], in_=osb)

    nc.compile()
    _NC_CACHE = nc
    return _NC_CACHE


def _prep_inputs(x, weights, bias_param):
    x = np.asarray(x, dtype=np.float32)
    weights = np.asarray(weights, dtype=np.float32)
    bias_param = np.asarray(bias_param, dtype=np.float32)

    # [o, i, k] -> [it, p, k'*1024 + o] in bf16 (k' = k-1; T_0 folded out)
    w5 = weights.transpose(1, 2, 0)[:, 1:, :]  # [i, 7, o]
    w5 = np.ascontiguousarray(
        w5.reshape(N_ITILES, P, KORD * OUT_F).astype(ml_dtypes.bfloat16)
    )

    bias_eff = bias_param + weights[:, :, 0].sum(axis=1)  # T_0 == 1 fold
    bias_t = np.ascontiguousarray(bias_eff.reshape(N_OTILES, P).T)  # [128, 8]

    in_maps = []
    for c in range(N_CORES):
        x_c = np.ascontiguousarray(x[c * B_CORE : (c + 1) * B_CORE].T)  # [1024, 512]
        in_maps.append({"xT": x_c, "wT": w5, "biasT": bias_t})
    return in_maps


def _run(x, weights, bias_param, **spmd_kwargs):
    nc = _build()
    in_maps = _prep_inputs(x, weights, bias_param)
    res = run_bass_kernel_spmd(nc, in_maps, core_ids=list(range(N_CORES)), **spmd_kwargs)
    out = np.empty((BATCH, OUT_F), dtype=np.float32)
    for c in range(N_CORES):
        o = np.asarray(res.results[c]["outT"]).astype(np.float32)
        out[c * B_CORE : (c + 1) * B_CORE] = o.T
    return out, res


def kernel(x, weights, bias_param):
    out, _ = _run(x, weights, bias_param)
    return out
